# revision 23
# baseline (speedup 1.0000x reference)
"""GCN-LSTM regressor as a Bass/Tile kernel for 8 Trainium2 NeuronCores.

Math restructuring (exact, up to fp reassociation):
  The reference GCN is rank-2 in disguise:
    m  = A_hat @ x_bt          (over lines)         (B,T,L)
    h1 = relu(m[...,None] * W1) ;  xw2 = h1 @ W2
       = m+ * u+  +  m- * u-   with u+ = relu(W1)@W2, u- = relu(-W1)@W2
    h2 = relu(p[...,None]*u+ + q[...,None]*u-),  p = A_hat@m+, q = A_hat@m-
  so the (B,T,L,G) tensors never need to exist.

Sharding: data-parallel over B*L = 480 LSTM sequences -> 60 per core
  (core c: batch b=c//2, lines l0=(c%2)*60 .. +60).  All weights replicated.

Layout: everything feature-on-partition ("transposed") so the LSTM
  recurrence h_t -> gates_{t+1} needs no per-step transposes.  All feature
  dims are zero-padded to multiples of 128 (HL 500->512, 4H 2000->2048,
  G 500->512, head 3000->3072, 1000->1024) so every matmul runs a full
  (128,128) stationary tile with fast-weight-load; padded lanes stay
  exactly 0 through the whole network (biases pad to 0 and sigmoid(0)*0
  terms vanish).

Schedule: gates_x production for chunk c+1 is emitted inside the step
  loop of chunk c so the tensor engine never idles during the per-step
  activation tail (keeps the PE clock un-throttled).
"""

import sys

sys.path.insert(0, "/opt/trn_rl_repo")

import numpy as np
import ml_dtypes

import concourse.bass as bass
import concourse.mybir as mybir
import concourse.tile as tile
from concourse import bacc
from concourse.bass_utils import run_bass_kernel_spmd
from concourse.masks import make_identity

BF16 = ml_dtypes.bfloat16
FP8 = ml_dtypes.float8_e4m3
F32 = mybir.dt.float32
BF = mybir.dt.bfloat16
F8 = mybir.dt.float8e4
WHH_SCALE = 512.0
AF = mybir.ActivationFunctionType
ALU = mybir.AluOpType

B, T, L, G, HL, OUT = 4, 192, 120, 500, 500, 24
_IDENT8 = None  # set below after P is defined
NCORES = 8
NLOC = 60  # lines per core
CH = 8  # LSTM steps per production chunk
NCH = T // CH  # 24 chunks
P = 128  # tile edge
KT = 4  # 512 = 4 k-tiles of 128
MT = 16  # 2048 = 16 m-tiles of 128
HLP, H4P, GP = 512, 2048, 512
F1, F2, F3 = 3072, 1024, 3072
NPOS = CH * NLOC  # 480 positions per chunk
_IDENT8 = np.eye(P, dtype=np.float32).astype(ml_dtypes.float8_e4m3)


def _build_program():
    nc = bacc.Bacc(
        "TRN2",
        target_bir_lowering=False,
        debug=False,
        enable_asserts=True,
        num_devices=NCORES,
    )

    xb = nc.declare_dram_parameter("xb", [2, 96, L], F32, isOutput=False)
    a_hat = nc.declare_dram_parameter("a_hat", [L, L], F32, isOutput=False)
    uu = nc.declare_dram_parameter("uu", [2, GP], BF, isOutput=False)
    wih_t = nc.declare_dram_parameter("wih_t", [P, 2, 2, H4P], F8, isOutput=False)
    ident8_d = nc.declare_dram_parameter("ident8", [P, P], F8, isOutput=False)
    whh_t = nc.declare_dram_parameter("whh_t", [P, KT, H4P], F8, isOutput=False)
    bias16 = nc.declare_dram_parameter("bias16", [P, MT], F32, isOutput=False)
    wh1 = nc.declare_dram_parameter("wh1", [P, 4, F1], BF, isOutput=False)
    wh2 = nc.declare_dram_parameter("wh2", [P, F1 // P, F2], BF, isOutput=False)
    wh3 = nc.declare_dram_parameter("wh3", [P, F2 // P, F3], BF, isOutput=False)
    wh4 = nc.declare_dram_parameter("wh4", [P, F3 // P, OUT], BF, isOutput=False)
    bh1s = nc.declare_dram_parameter("bh1s", [P, F1 // P], F32, isOutput=False)
    bh2s = nc.declare_dram_parameter("bh2s", [P, F2 // P], F32, isOutput=False)
    bh3s = nc.declare_dram_parameter("bh3s", [P, F3 // P], F32, isOutput=False)
    bh4s = nc.declare_dram_parameter("bh4s", [OUT, 1], F32, isOutput=False)
    out = nc.declare_dram_parameter("out", [OUT, NLOC], F32, isOutput=True)

    # per-core DRAM scratch for p/q (t-major so chunks slice rows)
    p_dram = nc.dram_tensor("p_dram", [T, L], BF)
    q_dram = nc.dram_tensor("q_dram", [T, L], BF)

    with tile.TileContext(nc) as tc:
        with (
            tc.tile_pool(name="const", bufs=1) as constp,
            tc.tile_pool(name="state", bufs=1) as statep,
            tc.tile_pool(name="headw", bufs=1) as headwp,
        ):
            # ---- constants ----
            a_sb = constp.tile([L, L], F32)
            nc.sync.dma_start(out=a_sb, in_=a_hat[:, :])
            uu_sb = constp.tile([2, GP], BF)
            nc.sync.dma_start(out=uu_sb, in_=uu[:, :])
            wih_sb = constp.tile([P, 2, 2, H4P], F8)
            nc.sync.dma_start(out=wih_sb, in_=wih_t[:, :, :, :])
            ident8 = constp.tile([P, P], F8)
            nc.sync.dma_start(out=ident8, in_=ident8_d[:, :])
            whh_sb = constp.tile([P, KT, H4P], F8)
            nc.sync.dma_start(out=whh_sb, in_=whh_t[:, :, :])
            b16_sb = constp.tile([P, MT], F32)
            nc.sync.dma_start(out=b16_sb, in_=bias16[:, :])
            ident = constp.tile([128, 128], F32)
            make_identity(nc, ident)

            # resident head weights (wh2/wh3 streamed in the head phase)
            wh1_sb = headwp.tile([P, 4, F1], BF)
            nc.sync.dma_start(out=wh1_sb, in_=wh1[:, :, :])
            wh4_sb = headwp.tile([P, F3 // P, OUT], BF)
            nc.sync.dma_start(out=wh4_sb, in_=wh4[:, :, :])
            bh1_sb = headwp.tile([P, F1 // P], F32)
            nc.sync.dma_start(out=bh1_sb, in_=bh1s[:, :])
            bh2_sb = headwp.tile([P, F2 // P], F32)
            nc.sync.dma_start(out=bh2_sb, in_=bh2s[:, :])
            bh3_sb = headwp.tile([P, F3 // P], F32)
            nc.sync.dma_start(out=bh3_sb, in_=bh3s[:, :])
            bh4_sb = headwp.tile([OUT, 1], F32)
            nc.sync.dma_start(out=bh4_sb, in_=bh4s[:, :])

            # ---- LSTM state ----
            hT = statep.tile([P, KT, NLOC], BF)
            ctg = statep.tile([P, 2, 4, NLOC], BF)  # [:,0]=c  [:,1]=tanh(g)
            nc.vector.memset(hT, 0.0)
            nc.vector.memset(ctg, 0.0)

            # ================= GCN (tiny) =================
            with (
                tc.tile_pool(name="gcn", bufs=2) as gcnp,
                tc.tile_pool(name="gcn1", bufs=1) as gcn1p,
                tc.tile_pool(name="gcn_ps", bufs=2, space="PSUM") as gcnps,
            ):
                xT_sb = gcn1p.tile([L, T], F32)
                for i in range(2):
                    xt = gcnp.tile([96, L], F32, tag="xt")
                    nc.sync.dma_start(out=xt, in_=xb[i])
                    xT_ps = gcnps.tile([L, 96], F32, tag="tp")
                    nc.tensor.transpose(xT_ps, xt, ident[:96, :96])
                    nc.scalar.copy(xT_sb[:, i * 96 : (i + 1) * 96], xT_ps)
                mT_ps = gcnps.tile([L, T], F32, tag="mm")
                nc.tensor.matmul(mT_ps, lhsT=a_sb, rhs=xT_sb, start=True, stop=True)
                mp_sb = gcn1p.tile([L, T], F32)
                mm_sb = gcn1p.tile([L, T], F32)
                nc.scalar.activation(mp_sb, mT_ps, AF.Relu)
                nc.scalar.activation(mm_sb, mT_ps, AF.Relu, scale=-1.0)
                for src, dst in ((mp_sb, p_dram), (mm_sb, q_dram)):
                    rT_ps = gcnps.tile([L, T], F32, tag="mm")
                    nc.tensor.matmul(rT_ps, lhsT=a_sb, rhs=src, start=True, stop=True)
                    rT_sb = gcnp.tile([L, T], F32, tag="rt")
                    nc.scalar.copy(rT_sb, rT_ps)
                    for i in range(2):
                        r_ps = gcnps.tile([96, L], F32, tag="tp2")
                        nc.tensor.transpose(
                            r_ps, rT_sb[:, i * 96 : (i + 1) * 96], ident[:L, :L]
                        )
                        r_sb = gcnp.tile([96, L], BF, tag="rsb")
                        nc.scalar.copy(r_sb, r_ps)
                        nc.sync.dma_start(out=dst[i * 96 : (i + 1) * 96, :], in_=r_sb)

            # ============ production + LSTM ============
            with (
                tc.tile_pool(name="pq", bufs=3) as pqp,
                tc.tile_pool(name="h2", bufs=3) as h2p,
                tc.tile_pool(name="gx", bufs=2) as gxp,
                tc.tile_pool(name="ltmp", bufs=3) as ltp,
                tc.tile_pool(name="h2_ps", bufs=1, space="PSUM") as h2ps,
                tc.tile_pool(name="gx_ps", bufs=3, space="PSUM") as gxps,
                tc.tile_pool(name="rec_ps", bufs=1, space="PSUM") as recps,
            ):
                h2_tiles = [None] * NCH
                gx_tiles = [None] * NCH

                pq_tiles = [None] * NCH

                def produce_pq(c):
                    pq = pqp.tile([2, CH, NLOC], BF, tag="pq", name="pq")
                    nc.sync.dma_start(
                        out=pq[0:1], in_=p_dram[c * CH : (c + 1) * CH, 0:NLOC][None]
                    )
                    nc.sync.dma_start(
                        out=pq[1:2], in_=q_dram[c * CH : (c + 1) * CH, 0:NLOC][None]
                    )
                    pq_tiles[c] = pq

                def produce_h2(c):
                    """h2T = relu(u+ p + u- q) for chunk c (fp8, x256)."""
                    pq = pq_tiles[c]
                    h2 = h2p.tile([P, 2, 2, NPOS], F8, tag="h2")
                    for gt in range(KT):
                        h2_ps = h2ps.tile([P, NPOS], F32, tag="h2ps")
                        nc.tensor.matmul(
                            h2_ps,
                            lhsT=uu_sb[:, gt * P : (gt + 1) * P],
                            rhs=pq,
                            start=True,
                            stop=True,
                        )
                        nc.scalar.activation(h2[:, gt // 2, gt % 2], h2_ps, AF.Relu)
                    h2_tiles[c] = h2

                def produce_gx_mtile(c, m):
                    """one m-tile of gxT = W_ih @ h2 + bias for chunk c."""
                    if m == 0:
                        gx_tiles[c] = gxp.tile([P, MT, NPOS], BF, tag="gx", name="gx")
                    gx = gx_tiles[c]
                    h2 = h2_tiles[c]
                    g_ps = gxps.tile([P, NPOS], F32, tag="gps")
                    for kd in range(2):
                        nc.tensor.matmul(
                            g_ps,
                            lhsT=wih_sb[:, kd, :, m * P : (m + 1) * P],
                            rhs=h2[:, kd],
                            start=(kd == 0),
                            stop=(kd == 1),
                            perf_mode=mybir.MatmulPerfMode.DoubleRow,
                        )
                    # descale h2's x256 and add the x512-scaled bias; gx is
                    # stored x512 so it can be injected into the scaled
                    # recurrent psum by an identity matmul.  Alternate the
                    # copy between the scalar and vector engines.
                    if m % 2 == 0:
                        nc.scalar.activation(
                            gx[:, m],
                            g_ps,
                            AF.Identity,
                            bias=b16_sb[:, m : m + 1],
                            scale=1.0 / 256.0,
                        )
                    else:
                        nc.vector.tensor_scalar(
                            out=gx[:, m],
                            in0=g_ps,
                            scalar1=1.0 / 256.0,
                            scalar2=b16_sb[:, m : m + 1],
                            op0=ALU.mult,
                            op1=ALU.add,
                        )

                # prologue: chunk 0 production runs un-overlapped
                produce_pq(0)
                produce_pq(1)
                produce_pq(2)
                produce_h2(0)
                produce_h2(1)
                for m in range(MT):
                    produce_gx_mtile(0, m)

                def rec_gate(gate, rp, gx, s):
                    """4 m-tiles x (4 whh k-matmuls + gx inject) for one gate."""
                    for mi in range(4):
                        m = gate * 4 + mi
                        for k in range(KT):
                            nc.tensor.matmul(
                                rp[:, m, 0:NLOC],
                                lhsT=whh_sb[:, k, m * P : (m + 1) * P],
                                rhs=hT[:, k],
                                start=(k == 0),
                                stop=False,
                            )
                        nc.tensor.matmul(
                            rp[:, m, 0:NLOC],
                            lhsT=ident8,
                            rhs=gx[:, m, s * NLOC : (s + 1) * NLOC],
                            start=False,
                            stop=True,
                        )

                # device gate order: 0=g 1=f 2=i 3=o (one PSUM bank each)
                DS = 1.0 / WHH_SCALE
                for c in range(NCH):
                    gx = gx_tiles[c]
                    for s in range(CH):
                        # [128, 16 m-tiles, 128] f32 = exactly 4 banks;
                        # slots 0-3 bank0 (g), 4-7 bank1 (f), 8-11 bank2 (i),
                        # 12-15 bank3 (o)
                        rp = recps.tile([P, MT, 128], F32, tag="rec", name="rec")
                        rec_gate(0, rp, gx, s)
                        nc.scalar.activation(
                            ctg[:, 1], rp[:, 0:4, 0:NLOC], AF.Tanh, scale=DS
                        )
                        rec_gate(1, rp, gx, s)
                        rec_gate(2, rp, gx, s)
                        sfi = ltp.tile([P, 2, 4, NLOC], BF, tag="sfi")
                        nc.scalar.activation(
                            sfi.rearrange("p a b n -> p (a b) n"),
                            rp[:, 4:12, 0:NLOC],
                            AF.Sigmoid,
                            scale=DS,
                        )
                        t12 = ltp.tile([P, 2, 4, NLOC], BF, tag="t12")
                        nc.vector.tensor_tensor(
                            t12.rearrange("p a b n -> p (a b n)"),
                            sfi.rearrange("p a b n -> p (a b n)"),
                            ctg.rearrange("p a b n -> p (a b n)"),
                            op=ALU.mult,
                        )
                        nc.vector.tensor_tensor(
                            ctg[:, 0].rearrange("p b n -> p (b n)"),
                            t12[:, 0].rearrange("p b n -> p (b n)"),
                            t12[:, 1].rearrange("p b n -> p (b n)"),
                            op=ALU.add,
                        )
                        tc_ = ltp.tile([P, 4, NLOC], BF, tag="tc")
                        nc.scalar.activation(tc_, ctg[:, 0], AF.Tanh)
                        rec_gate(3, rp, gx, s)
                        so = ltp.tile([P, 4, NLOC], BF, tag="so")
                        nc.scalar.activation(
                            so, rp[:, 12:16, 0:NLOC], AF.Sigmoid, scale=DS
                        )
                        # h slice-by-slice so next step's k0 matmul starts
                        # before the whole h update finishes
                        for k in range(KT):
                            nc.vector.tensor_tensor(
                                hT[:, k], so[:, k], tc_[:, k], op=ALU.mult
                            )
                        # gap fillers AFTER the critical tail so their
                        # copies queue behind it on ACT/DVE, not ahead
                        if c + 1 < NCH:
                            produce_gx_mtile(c + 1, 2 * s)
                            produce_gx_mtile(c + 1, 2 * s + 1)
                        if s == CH - 1 and c + 2 < NCH:
                            if c + 3 < NCH:
                                produce_pq(c + 3)
                            produce_h2(c + 2)

            # ================= head =================
            with (
                tc.tile_pool(name="hd", bufs=3) as hdp,
                tc.tile_pool(name="hd1", bufs=1) as hd1p,
                tc.tile_pool(name="hd_ps", bufs=4, space="PSUM") as hdps,
            ):
                z1 = hd1p.tile([P, F1 // P, NLOC], BF)
                for m in range(F1 // P):
                    ps = hdps.tile([P, NLOC], F32, tag="zps")
                    for k in range(4):
                        nc.tensor.matmul(
                            ps,
                            lhsT=wh1_sb[:, k, m * P : (m + 1) * P],
                            rhs=hT[:, k],
                            start=(k == 0),
                            stop=(k == 3),
                        )
                    nc.scalar.activation(
                        z1[:, m], ps, AF.Relu, bias=bh1_sb[:, m : m + 1]
                    )
                z2 = hd1p.tile([P, F2 // P, NLOC], BF)
                for m in range(F2 // P):
                    w2t = hdp.tile([P, F1 // P, P], BF, tag="w2t")
                    nc.sync.dma_start(out=w2t, in_=wh2[:, :, m * P : (m + 1) * P])
                    ps = hdps.tile([P, NLOC], F32, tag="zps")
                    for k in range(F1 // P):
                        nc.tensor.matmul(
                            ps,
                            lhsT=w2t[:, k],
                            rhs=z1[:, k],
                            start=(k == 0),
                            stop=(k == F1 // P - 1),
                        )
                    nc.scalar.activation(
                        z2[:, m], ps, AF.Relu, bias=bh2_sb[:, m : m + 1]
                    )
                z3 = hd1p.tile([P, F3 // P, NLOC], BF)
                for m in range(F3 // P):
                    w3t = hdp.tile([P, F2 // P, P], BF, tag="w3t")
                    nc.sync.dma_start(out=w3t, in_=wh3[:, :, m * P : (m + 1) * P])
                    ps = hdps.tile([P, NLOC], F32, tag="zps")
                    for k in range(F2 // P):
                        nc.tensor.matmul(
                            ps,
                            lhsT=w3t[:, k],
                            rhs=z2[:, k],
                            start=(k == 0),
                            stop=(k == F2 // P - 1),
                        )
                    nc.scalar.activation(
                        z3[:, m], ps, AF.Relu, bias=bh3_sb[:, m : m + 1]
                    )
                ps4 = hdps.tile([OUT, NLOC], F32, tag="z4")
                for k in range(F3 // P):
                    nc.tensor.matmul(
                        ps4,
                        lhsT=wh4_sb[:, k],
                        rhs=z3[:, k],
                        start=(k == 0),
                        stop=(k == F3 // P - 1),
                    )
                y_sb = hd1p.tile([OUT, NLOC], F32)
                nc.scalar.activation(y_sb, ps4, AF.Sigmoid, bias=bh4_sb[:, 0:1])
                nc.sync.dma_start(out=out[:, :], in_=y_sb)

    nc.compile()
    return nc


_PROG = None
_LAST_RESULTS = None


def _get_program():
    global _PROG
    if _PROG is None:
        _PROG = _build_program()
    return _PROG


GATE_PERM = (2, 1, 0, 3)  # device gate order [g, f, i, o] from pytorch [i, f, g, o]


def _pad_gates(w, pad_in, pad_unit):
    """(4*HL, K) -> (4*pad_unit, pad_in), gate blocks permuted to GATE_PERM."""
    H4_, K_ = w.shape
    hl = H4_ // 4
    out = np.zeros((4 * pad_unit, pad_in), w.dtype)
    for g in range(4):
        src = GATE_PERM[g]
        out[g * pad_unit : g * pad_unit + hl, :K_] = w[src * hl : (src + 1) * hl]
    return out


def _kstack(wT, p=P):
    """(K, M) -> (p, K//p, M) partition-major for SBUF staging."""
    K_, M_ = wT.shape
    return np.ascontiguousarray(wT.reshape(K_ // p, p, M_).transpose(1, 0, 2))


def _prep(W1, W2, W_ih, W_hh, b_ih, b_hh, Wh1, bh1, Wh2, bh2, Wh3, bh3, Wh4, bh4):
    f = np.float32
    u_plus = np.maximum(W1[0], 0) @ W2  # (G,)
    u_minus = np.maximum(-W1[0], 0) @ W2
    uu = np.zeros((2, GP), f)
    uu[0, :G] = u_plus
    uu[1, :G] = u_minus
    uu = (uu * np.float32(256.0)).astype(BF16)

    # W_ih: (2000, 500) -> padded (2048, 512) -> T -> (512, 2048); x512 fp8
    # in DoubleRow layout [p, kd, plane, m] with k = kd*256 + plane*128 + p
    wih_p = _pad_gates(W_ih, GP, HLP) * np.float32(WHH_SCALE)  # (2048, 512)
    wih_t = np.ascontiguousarray(
        wih_p.T.reshape(2, 2, P, H4P).transpose(2, 0, 1, 3)
    ).astype(FP8)
    whh_p = _pad_gates(W_hh, HLP, HLP) * np.float32(WHH_SCALE)  # (2048, 512)
    whh_t = _kstack(np.ascontiguousarray(whh_p.T)).astype(FP8)
    bias = np.zeros(H4P, f)
    bb = (b_ih + b_hh).astype(f)
    for g in range(4):
        src = GATE_PERM[g]
        bias[g * HLP : g * HLP + HL] = bb[src * HL : (src + 1) * HL]
    bias16 = np.ascontiguousarray(bias.reshape(MT, P).T) * np.float32(
        WHH_SCALE
    )  # (128,16), x512

    def pad2(w, r, c):
        o = np.zeros((r, c), f)
        o[: w.shape[0], : w.shape[1]] = w
        return o

    wh1 = _kstack(pad2(Wh1, HLP, F1)).astype(BF16)
    wh2 = _kstack(pad2(Wh2, F1, F2)).astype(BF16)
    wh3 = _kstack(pad2(Wh3, F2, F3)).astype(BF16)
    wh4 = _kstack(pad2(Wh4, F3, OUT)).astype(BF16)
    bh1s = np.ascontiguousarray(pad2(bh1[None], 1, F1)[0].reshape(F1 // P, P).T)
    bh2s = np.ascontiguousarray(pad2(bh2[None], 1, F2)[0].reshape(F2 // P, P).T)
    bh3s = np.ascontiguousarray(pad2(bh3[None], 1, F3)[0].reshape(F3 // P, P).T)
    bh4s = np.ascontiguousarray(bh4.astype(f).reshape(OUT, 1))
    return uu, wih_t, whh_t, bias16, wh1, wh2, wh3, wh4, bh1s, bh2s, bh3s, bh4s


def kernel(
    x,
    A_hat,
    W1,
    W2,
    W_ih,
    W_hh,
    b_ih,
    b_hh,
    Wh1,
    bh1,
    Wh2,
    bh2,
    Wh3,
    bh3,
    Wh4,
    bh4,
):
    f = np.float32
    x = np.asarray(x, f)
    nc = _get_program()
    args = [
        np.asarray(a, f)
        for a in (
            W1,
            W2,
            W_ih,
            W_hh,
            b_ih,
            b_hh,
            Wh1,
            bh1,
            Wh2,
            bh2,
            Wh3,
            bh3,
            Wh4,
            bh4,
        )
    ]
    uu, wih_t, whh_t, bias16, wh1, wh2, wh3, wh4, bh1s, bh2s, bh3s, bh4s = _prep(*args)
    a_hat = np.ascontiguousarray(np.asarray(A_hat, f))

    # odd cores handle lines 60..119: roll lines so theirs sit at 0..59
    # (the GCN is permutation-equivariant when A_hat is permuted to match)
    a_roll = np.ascontiguousarray(np.roll(np.roll(a_hat, -NLOC, 0), -NLOC, 1))
    in_maps = []
    for c in range(NCORES):
        b = c // 2
        if c % 2 == 0:
            xc, ac = x[b], a_hat
        else:
            xc, ac = np.roll(x[b], -NLOC, axis=-1), a_roll
        in_maps.append(
            {
                "xb": np.ascontiguousarray(xc.reshape(2, 96, L)),
                "a_hat": ac,
                "uu": uu,
                "wih_t": wih_t,
                "ident8": _IDENT8,
                "whh_t": whh_t,
                "bias16": bias16,
                "wh1": wh1,
                "wh2": wh2,
                "wh3": wh3,
                "wh4": wh4,
                "bh1s": bh1s,
                "bh2s": bh2s,
                "bh3s": bh3s,
                "bh4s": bh4s,
            }
        )

    global _LAST_RESULTS
    _LAST_RESULTS = run_bass_kernel_spmd(nc, in_maps, list(range(NCORES)))
    res = _LAST_RESULTS.results
    y = np.zeros((B, OUT, L), f)
    for c in range(NCORES):
        b = c // 2
        l0 = (c % 2) * NLOC
        y[b, :, l0 : l0 + NLOC] = res[c]["out"]
    return y
